# revision 48
# baseline (speedup 1.0000x reference)
"""Ernie4 decoder layer (RMSNorm + GQA attention + shared expert + 16-expert
top-2 MoE) on 8 Trainium2 NeuronCores.

v2 — fp16 data path everywhere except the router (which must reproduce the
reference top-2 selection exactly; margins are ~3e-5 so it stays fp32 and is
computed locally per core before the AllGather):
  - Attention: head-parallel (2 q-heads + 1 kv-head per core), fp16 QKV /
    scores / probs / o_proj with causal-block skipping; fp16 ReduceScatter.
  - Router: fp32 logits on each core's own 128 tokens; W+sel AllGathered in a
    tiny fp32 collective that precedes the fp16 x AllGather so the token-list
    build overlaps it.
  - Shared expert: intermediate-sharded (IS/8 per core) fp16, output seeds
    the MoE combine buffer.
  - MoE: expert-parallel (2 experts per core), token lists via
    triangular-matmul prefix ranks, indirect-DMA gather/scatter-add in fp16,
    fp16 ReduceScatter for the combine.
"""
import sys
sys.path.insert(0, "/opt/trn_rl_repo")

import numpy as np

import concourse.bass as bass
import concourse.bacc as bacc
import concourse.tile as tile
import concourse.mybir as mybir
from concourse import bass_utils
from concourse.masks import make_identity
from concourse.tile import add_dep_helper

dt = mybir.dt
F32 = dt.float32
F32R = dt.float32r
F16 = dt.float16
I32 = dt.int32
BF16 = dt.bfloat16
AF = mybir.ActivationFunctionType
ALU = mybir.AluOpType
AX = mybir.AxisListType

T, H, NH, NKV, D = 1024, 2048, 16, 4, 128
E, I, IS = 16, 1024, 2048
EPS = 1e-6
THETA = 10000.0
NCN = 8
P = 128
TB = T // P            # 8 token blocks
HC = H // P            # 16 hidden chunks
IP = I // P            # 8 expert-intermediate chunks
SP = IS // NCN // P    # 2 shared-intermediate chunks per core
CAP = 256              # per-expert token capacity
BIG = 1.0e6            # OOB sentinel
NEG = -1e9
S64 = P // 2
PERM_ALL = np.concatenate(
    [np.concatenate([np.arange(c * S64, (c + 1) * S64),
                     T // 2 + np.arange(c * S64, (c + 1) * S64)])
     for c in range(NCN)])


def _emit(nc, tc):
    ex = {}
    for name, shape, d in [
        ("hid", [T, H], F32), ("hid_slice", [P, H], F32),
        ("wq_s", [H, 2 * D], F16), ("wk_s", [H, D], F16), ("wv_s", [H, D], F16),
        ("wo_s", [2 * D, H], F16),
        ("cosq", [D, T], F32), ("sinq", [D, T], F32),
        ("cosk", [D, T], F32), ("sink", [D, T], F32),
        ("perm", [P, P], F32),
        ("diag_mask", [P, P], BF16),
        ("gate_wT", [H, E], F32), ("gate_b", [P, E], F32),
        ("emask01", [P, 2 * E], F32),
        ("ws_g", [H, SP * P], F16), ("ws_u", [H, SP * P], F16),
        ("ws_d", [SP * P, H], F16),
        ("we_g", [2, H, I], F16), ("we_u", [2, H, I], F16),
        ("we_d", [2, I, H], F16),
        ("identr_in", [P, P], F32), ("identh_in", [P, P], F16),
        ("ut_in", [P, P], F32), ("ut_h", [P, P], F16),
        ("slb_in", [8, TB * P], F32),
        ("slot_iota", [P, CAP], F32), ("tokid2", [P, 2 * TB], F16),
    ]:
        ex[name] = nc.dram_tensor(name, shape, d, kind="ExternalInput").ap()
    out_slice = nc.dram_tensor("out_slice", [P, H], F32, kind="ExternalOutput").ap()
    res_slice = nc.dram_tensor("res_slice", [P, H], F32, kind="ExternalOutput").ap()
    dbg_w = nc.dram_tensor("dbg_w", [P, E], F32, kind="ExternalOutput").ap()

    with tc.tile_pool(name="persist", bufs=1) as pp, \
         tc.tile_pool(name="dram", bufs=1, space="DRAM") as dram:
        rs_in2 = [dram.tile([T // 2, H], F16, name=f"rs_in{i}")
                  for i in range(2)]
        rs_out2 = [dram.tile([P // 2, H], F16, name=f"rs_out{i}")
                   for i in range(2)]
        agx_in = dram.tile([P, H], F16)
        x_tm = dram.tile([T, H], F16, addr_space="Shared")
        agw_in = dram.tile([P, 2 * E], F16)
        w_all = dram.tile([T, 2 * E], F16, addr_space="Shared")
        warm_in = dram.tile([8, 8], F16)
        warm_out = dram.tile([64, 8], F16, addr_space="Shared")
        rs2_in = dram.tile([T, H], F16)
        rs2_out = dram.tile([P, H], F16)

        ident = pp.tile([P, P], F32)
        make_identity(nc, ident[:])
        identr = pp.tile([P, P], F32R)
        nc.sync.dma_start(identr[:], ex["identr_in"][:].bitcast(F32R))
        identh = pp.tile([P, P], F16)
        nc.sync.dma_start(identh[:], ex["identh_in"][:])
        hid_sl2 = [pp.tile([P // 2, H], F32, tag=f"hidsl{i}",
                           name=f"hidsl{i}") for i in range(2)]
        for i in range(2):
            nc.sync.dma_start(
                hid_sl2[i][:],
                ex["hid_slice"][i * (P // 2):(i + 1) * (P // 2), :])
        eps_t = pp.tile([P, 1], F32)
        nc.vector.memset(eps_t[:], EPS)
        emask01 = pp.tile([P, 2 * E], F32)
        nc.sync.dma_start(emask01[:], ex["emask01"][:])
        wz = pp.tile([8, 8], F16)
        nc.vector.memset(wz[:], 0.0)
        nc.sync.dma_start(warm_in[:], wz[:])
        nc.gpsimd.collective_compute(
            "AllGather", ALU.bypass, ins=[warm_in.opt()],
            outs=[warm_out.opt()], replica_groups=[list(range(NCN))])
        # per-expert token lists live in SBUF end-to-end (built by the
        # matmul-based inverse permutation in phase E, consumed in F)
        idx_sb2 = [[pp.tile([P, 1], I32, tag=f"idx{ei}{k}",
                            name=f"idx{ei}{k}") for k in range(2)]
                   for ei in range(2)]
        # shared-expert weights are pure inputs: load them from t=0 so the
        # post-AllGather phase never waits on weight DMAs
        wsg_sb = pp.tile([P, HC * SP * P], F16)
        wsu_sb = pp.tile([P, HC * SP * P], F16)
        for t_, s_ in [(wsg_sb, "ws_g"), (wsu_sb, "ws_u")]:
            nc.sync.dma_start(
                t_[:].rearrange("p (hc m) -> p hc m", hc=HC),
                ex[s_][:].rearrange("(hc p) m -> p hc m", p=P))
        wsd_sb = [pp.tile([P, H], F16, tag=f"wsd{sp}", name=f"wsd{sp}")
                  for sp in range(SP)]
        for sp in range(SP):
            nc.sync.dma_start(wsd_sb[sp][:],
                              ex["ws_d"][sp * P:(sp + 1) * P, :])

        # ======== Phases A-C: attention (fp16) ========
        with tc.tile_pool(name="pab", bufs=1) as pab:
            qT = [pab.tile([P, T], F16, tag=f"qT{j}", name=f"qT{j}")
                  for j in range(2)]
            kT = pab.tile([P, T], F16)
            vT = pab.tile([P, T], F16)
            v_tm = [pab.tile([P, D], F16, tag=f"vtm{b}", name=f"vtm{b}")
                    for b in range(TB)]
            oT = [pab.tile([P, T], F16, tag=f"oT{j}", name=f"oT{j}")
                  for j in range(2)]

            # ---- A: norm + transpose + QKV + rope ----
            with tc.tile_pool(name="pa", bufs=1) as pa, \
                 tc.tile_pool(name="pa2", bufs=3) as pa2:
                cosq = pa.tile([D, T], F32)
                sinq = pa.tile([D, T], F32)
                cosk = pa.tile([D, T], F32)
                sink = pa.tile([D, T], F32)
                for t_, s_ in [(cosq, "cosq"), (sinq, "sinq"),
                               (cosk, "cosk"), (sink, "sink")]:
                    nc.sync.dma_start(t_[:], ex[s_][:])
                permr = pa.tile([P, P], F32R)
                nc.sync.dma_start(permr[:], ex["perm"][:].bitcast(F32R))
                wq_sb = pa.tile([P, HC * 2 * D], F16)
                wk_sb = pa.tile([P, HC * D], F16)
                wv_sb = pa.tile([P, HC * D], F16)
                for t_, s_, m in [(wq_sb, "wq_s", 2 * D), (wk_sb, "wk_s", D),
                                  (wv_sb, "wv_s", D)]:
                    nc.sync.dma_start(
                        t_[:].rearrange("p (hc m) -> p hc m", hc=HC),
                        ex[s_][:].rearrange("(hc p) m -> p hc m", p=P))

                dump = pa.tile([P, H], F32)
                qraw = [pa.tile([P, T], F32R, tag=f"qraw{j}", name=f"qraw{j}")
                        for j in range(2)]
                kraw = pa.tile([P, T], F32R)
                with tc.tile_pool(name="psA1", bufs=3, space="PSUM") as psA1, \
                     tc.tile_pool(name="psA2", bufs=3, space="PSUM") as psA2:
                    for n in range(2):
                        x0T = [pa.tile([P, 512], F16, tag=f"x0T{hc}",
                                       name=f"x0T{hc}_{n}") for hc in range(HC)]
                        for bb in range(TB // 2):
                            b = n * (TB // 2) + bb
                            hidb = pa2.tile([P, H], F32, tag="hidb", bufs=2)
                            nc.sync.dma_start(hidb[:],
                                              ex["hid"][b * P:(b + 1) * P, :])
                            ssum = pa2.tile([P, 1], F32, tag="ssum")
                            nc.scalar.activation(dump[:], hidb[:], AF.Square,
                                                 accum_out=ssum[:, :1])
                            rms = pa2.tile([P, 1], F32, tag="rms")
                            nc.scalar.activation(rms[:], ssum[:],
                                                 AF.Sqrt, bias=eps_t[:, :1],
                                                 scale=1.0 / H)
                            inv = pa2.tile([P, 1], F32, tag="inv")
                            nc.vector.reciprocal(inv[:], rms[:])
                            x0b = pa2.tile([P, H], F16, tag="x0b", bufs=2)
                            nc.vector.tensor_scalar_mul(x0b[:], hidb[:],
                                                        inv[:, :1])
                            for hc in range(HC):
                                tp = psA1.tile([P, P], F16, tag="tpA")
                                nc.tensor.transpose(
                                    tp[:], x0b[:, hc * P:(hc + 1) * P],
                                    identh[:])
                                nc.vector.tensor_copy(
                                    x0T[hc][:, bb * P:(bb + 1) * P], tp[:])

                        def proj(w_sb, m, c0, dst, n=n, x0T=x0T, fp16=False):
                            ps = psA2.tile([P, 512], F32, tag="psQKV",
                                           name="psQKV")
                            for hc in range(HC):
                                nc.tensor.matmul(
                                    ps[:],
                                    w_sb[:, hc * m + c0:hc * m + c0 + P],
                                    x0T[hc][:],
                                    start=(hc == 0), stop=(hc == HC - 1))
                            nc.vector.tensor_copy(
                                dst[:, n * 512:(n + 1) * 512], ps[:])
                        proj(wq_sb, 2 * D, 0, qraw[0])
                        proj(wq_sb, 2 * D, D, qraw[1])
                        proj(wk_sb, D, 0, kraw)
                        proj(wv_sb, D, 0, vT, fp16=True)

                with tc.tile_pool(name="psA3", bufs=2, space="PSUM") as psA3:
                    for src, dst, c_, s_ in [(qraw[0], qT[0], cosq, sinq),
                                             (qraw[1], qT[1], cosq, sinq),
                                             (kraw, kT, cosk, sink)]:
                        for n in range(2):
                            sl = slice(n * 512, (n + 1) * 512)
                            sw = psA3.tile([P, 512], F32, tag="psSW")
                            nc.tensor.matmul(sw[:], permr[:], src[:, sl],
                                             start=True, stop=True)
                            t1 = pa2.tile([P, 512], F32, tag="ropeT1")
                            nc.vector.tensor_mul(t1[:], src[:, sl], c_[:, sl])
                            t2 = pa2.tile([P, 512], F32, tag="ropeT2")
                            nc.vector.tensor_mul(t2[:], sw[:], s_[:, sl])
                            nc.vector.tensor_add(dst[:, sl], t1[:], t2[:])
                    for b in range(TB):
                        tp = psA3.tile([P, P], F16, tag="tpV")
                        nc.tensor.transpose(tp[:], vT[:, b * P:(b + 1) * P],
                                            identh[:])
                        nc.vector.tensor_copy(v_tm[b][:], tp[:])

            # ---- B+C: attention, token-half pipelined with the two
            # half ReduceScatters (RS1a hides under second-half compute) ----
            with tc.tile_pool(name="pb", bufs=1) as pb, \
                 tc.tile_pool(name="pb2", bufs=4) as pb2:
                dmask = pb.tile([P, P], BF16)
                nc.sync.dma_start(dmask[:], ex["diag_mask"][:])
                wo_sb = [pb.tile([P, H], F16, tag=f"wo{j}", name=f"wo{j}")
                         for j in range(2)]
                nc.sync.dma_start(wo_sb[0][:], ex["wo_s"][0:P, :])
                nc.sync.dma_start(wo_sb[1][:], ex["wo_s"][P:2 * P, :])

                attnT2 = [[pb.tile([P, T], F16, tag=f"attnT{h}{kc}",
                                   name=f"attnT{h}{kc}") for kc in range(TB)]
                          for h in range(2)]
                for h in range(2):
                    for kc in range(1, TB):
                        nc.vector.memset(attnT2[h][kc][:, 0:kc * P], 0.0)
                with tc.tile_pool(name="psB1", bufs=2, space="PSUM") as psB1, \
                     tc.tile_pool(name="psB2", bufs=2, space="PSUM") as psB2, \
                     tc.tile_pool(name="psB3", bufs=2, space="PSUM") as psB3, \
                     tc.tile_pool(name="psC", bufs=2, space="PSUM") as psC:
                    for half in range(2):
                        for qc in range(half * 4, half * 4 + 4):
                            cols = (qc + 1) * P
                            nsl = (cols + 511) // 512
                            for h in range(2):
                                prob = pb2.tile([P, T], F32, tag="prob")
                                for n in range(nsl):
                                    w_ = min(512, cols - n * 512)
                                    ps = psB1.tile([P, 512], F32, tag="psSC")
                                    nc.tensor.matmul(
                                        ps[:, :w_],
                                        qT[h][:, qc * P:(qc + 1) * P],
                                        kT[:, n * 512:n * 512 + w_],
                                        start=True, stop=True)
                                    d0 = qc * P - n * 512
                                    if 0 <= d0 < w_:
                                        if d0 > 0:
                                            nc.vector.tensor_copy(
                                                prob[:, n * 512:n * 512 + d0],
                                                ps[:, :d0])
                                        nc.vector.tensor_add(
                                            prob[:, qc * P:qc * P + P],
                                            ps[:, d0:d0 + P], dmask[:])
                                    else:
                                        nc.vector.tensor_copy(
                                            prob[:, n * 512:n * 512 + w_],
                                            ps[:, :w_])
                                mx = pb2.tile([P, 1], F32, tag="mx")
                                nc.vector.reduce_max(mx[:], prob[:, :cols],
                                                     axis=AX.X)
                                negm = pb2.tile([P, 1], F32, tag="negm")
                                nc.vector.tensor_scalar_mul(negm[:], mx[:],
                                                            -1.0)
                                ssum = pb2.tile([P, 1], F32, tag="esum")
                                probe_ = pb2.tile([P, T], F32, tag="probe")
                                nc.scalar.activation(
                                    probe_[:, :cols], prob[:, :cols],
                                    AF.Exp, bias=negm[:, :1],
                                    accum_out=ssum[:, :1])
                                rec = pb2.tile([P, 1], F32, tag="rec")
                                nc.vector.reciprocal(rec[:], ssum[:])
                                probS = pb2.tile([P, T], F16, tag="probS")
                                nc.scalar.activation(probS[:, :cols],
                                                     probe_[:, :cols],
                                                     AF.Copy,
                                                     scale=rec[:, :1])
                                for kc in range(qc + 1):
                                    tp = psB2.tile([P, P], F16, tag="tpB")
                                    nc.tensor.transpose(
                                        tp[:], probS[:, kc * P:(kc + 1) * P],
                                        identh[:])
                                    nc.vector.tensor_copy(
                                        attnT2[h][kc][:,
                                                      qc * P:(qc + 1) * P],
                                        tp[:])
                        sl = slice(half * 512, (half + 1) * 512)
                        kc_hi = 4 * half + 3
                        for h in range(2):
                            ps = psB3.tile([P, 512], F32, tag="psAV")
                            for kc in range(kc_hi + 1):
                                nc.tensor.matmul(ps[:], v_tm[kc][:],
                                                 attnT2[h][kc][:, sl],
                                                 start=(kc == 0),
                                                 stop=(kc == kc_hi))
                            if h == 0:
                                nc.vector.tensor_copy(oT[h][:, sl], ps[:])
                            else:
                                nc.scalar.activation(oT[h][:, sl], ps[:],
                                                     AF.Copy)
                        for tb_ in range(half * 4, half * 4 + 4):
                            ob = pb2.tile([P, H], F16, tag="ob", bufs=2)
                            for n in range(4):
                                pso = psC.tile([P, 512], F32, tag="psO")
                                for hp in range(2):
                                    nc.tensor.matmul(
                                        pso[:],
                                        oT[hp][:, tb_ * P:(tb_ + 1) * P],
                                        wo_sb[hp][:, n * 512:(n + 1) * 512],
                                        start=(hp == 0), stop=(hp == 1))
                                if n % 2 == 0:
                                    nc.vector.tensor_copy(
                                        ob[:, n * 512:(n + 1) * 512], pso[:])
                                else:
                                    nc.scalar.activation(
                                        ob[:, n * 512:(n + 1) * 512], pso[:],
                                        AF.Copy)
                            r0 = (tb_ - half * 4) * P
                            nc.sync.dma_start(rs_in2[half][r0:r0 + P, :],
                                              ob[:])
                        nc.gpsimd.collective_compute(
                            "ReduceScatter", ALU.add,
                            ins=[rs_in2[half].opt()],
                            outs=[rs_out2[half].opt()],
                            replica_groups=[list(range(NCN))])

        # ======== D: residual + norm + local fp32 router, per half ========
        S = P // 2
        with tc.tile_pool(name="pd", bufs=1) as pd, \
             tc.tile_pool(name="pd2", bufs=2) as pd2, \
             tc.tile_pool(name="psD", bufs=3, space="PSUM") as psD:
            gw_sb = pd.tile([P, HC * E], F32)
            nc.sync.dma_start(
                gw_sb[:].rearrange("p (hc e) -> p hc e", hc=HC),
                ex["gate_wT"][:].rearrange("(hc p) e -> p hc e", p=P))
            gate_b = pd.tile([P, E], F32)
            nc.sync.dma_start(gate_b[:], ex["gate_b"][:])
            for half in range(2):
                r0 = half * S
                attn_sl = pd.tile([S, H], F16, tag=f"attnsl{half}",
                                  name=f"attnsl{half}")
                nc.sync.dma_start(attn_sl[:], rs_out2[half][:])
                res_sb = pd.tile([S, H], F32, tag=f"ressb{half}",
                                 name=f"ressb{half}")
                dump2 = pd.tile([S, H], F32, tag=f"dump{half}",
                                name=f"dump{half}")
                ssum4 = pd.tile([S, 4], F32, tag=f"ssum4{half}",
                                name=f"ssum4{half}")
                for q in range(4):
                    sl = slice(q * 512, (q + 1) * 512)
                    nc.gpsimd.tensor_add(res_sb[:, sl],
                                         hid_sl2[half][:, sl],
                                         attn_sl[:, sl])
                    nc.scalar.activation(dump2[:, sl], res_sb[:, sl],
                                         AF.Square,
                                         accum_out=ssum4[:, q:q + 1])
                nc.sync.dma_start(res_slice[r0:r0 + S, :], res_sb[:])
                ssum2 = pd.tile([S, 2], F32, tag=f"ssum2{half}",
                                name=f"ssum2{half}")
                nc.vector.tensor_add(ssum2[:], ssum4[:, 0:2], ssum4[:, 2:4])
                ssum = pd.tile([S, 1], F32, tag=f"ssum{half}",
                               name=f"ssum{half}")
                nc.vector.tensor_add(ssum[:], ssum2[:, 0:1], ssum2[:, 1:2])
                rms = pd.tile([S, 1], F32, tag=f"rms{half}",
                              name=f"rms{half}")
                nc.scalar.activation(rms[:], ssum[:], AF.Sqrt,
                                     bias=eps_t[:S, :1], scale=1.0 / H)
                inv = pd.tile([S, 1], F32, tag=f"inv{half}",
                              name=f"inv{half}")
                nc.vector.reciprocal(inv[:], rms[:])
                x_sl_h = pd.tile([S, H], F16, tag=f"xslh{half}",
                                 name=f"xslh{half}")
                nc.vector.tensor_scalar_mul(x_sl_h[:], res_sb[:], inv[:, :1])
                nc.sync.dma_start(agx_in[r0:r0 + S, 0:H], x_sl_h[:])

                resT = pd.tile([P, HC * S], F32, tag=f"resT{half}",
                               name=f"resT{half}")
                for hc in range(HC):
                    tp = psD.tile([P, S], F32, tag="tpD")
                    nc.tensor.transpose(tp[:], res_sb[:, hc * P:(hc + 1) * P],
                                        ident[:S, :S])
                    nc.vector.tensor_copy(resT[:, hc * S:(hc + 1) * S], tp[:])
                lg_ps = psD.tile([S, E], F32, tag="lgps")
                for hc in range(HC):
                    nc.tensor.matmul(lg_ps[:], resT[:, hc * S:(hc + 1) * S],
                                     gw_sb[:, hc * E:(hc + 1) * E],
                                     start=(hc == 0), stop=(hc == HC - 1))
                sig = pd2.tile([S, E], F32, tag="sig")
                nc.scalar.activation(sig[:], lg_ps[:], AF.Sigmoid,
                                     scale=inv[:, :1])
                sb_ = pd2.tile([S, E], F32, tag="sb_")
                nc.vector.tensor_add(sb_[:], sig[:], gate_b[:S, :])
                mx = pd2.tile([S, 8], F32, tag="mx8")
                nc.vector.max(out=mx[:], in_=sb_[:])
                s1 = pd2.tile([S, E], F32, tag="s1")
                nc.vector.tensor_tensor(out=s1[:], in0=sb_[:],
                                        in1=mx[:, 0:1].to_broadcast([S, E]),
                                        op=ALU.is_equal)
                s2 = pd2.tile([S, E], F32, tag="s2")
                nc.vector.tensor_tensor(out=s2[:], in0=sb_[:],
                                        in1=mx[:, 1:2].to_broadcast([S, E]),
                                        op=ALU.is_equal)
                nc.vector.tensor_add(s1[:], s1[:], s2[:])
                sel_own = pd2.tile([S, E], F32, tag="sel_own")
                nc.vector.tensor_scalar_min(sel_own[:], s1[:], 1.0)
                wa = pd2.tile([S, E], F32, tag="wa")
                nc.vector.tensor_mul(wa[:], sel_own[:], sig[:])
                nrm = pd2.tile([S, 1], F32, tag="nrm")
                nc.vector.reduce_sum(nrm[:], wa[:], axis=AX.X)
                rec = pd2.tile([S, 1], F32, tag="recw")
                nc.vector.reciprocal(rec[:], nrm[:])
                w_tm = pd2.tile([S, E], F32, tag="wtm")
                nc.vector.tensor_scalar_mul(w_tm[:], wa[:], rec[:, :1])
                wsel_h = pd2.tile([S, 2 * E], F16, tag="wselh")
                nc.vector.tensor_copy(wsel_h[:, 0:E], w_tm[:])
                nc.vector.tensor_copy(wsel_h[:, E:2 * E], sel_own[:])
                nc.sync.dma_start(agw_in[r0:r0 + S, :], wsel_h[:])
                nc.sync.dma_start(dbg_w[r0:r0 + S, :], w_tm[:])

        cc_x = nc.gpsimd.collective_compute(
            "AllGather", ALU.bypass, ins=[agx_in.opt()], outs=[x_tm.opt()],
            replica_groups=[list(range(NCN))])
        cc_w = nc.gpsimd.collective_compute(
            "AllGather", ALU.bypass, ins=[agw_in.opt()], outs=[w_all.opt()],
            replica_groups=[list(range(NCN))])
        add_dep_helper(cc_w.ins, cc_x.ins, sync=True,
                       reason="AG_x (ready first) before AG_W")

        # ======== E: token lists from AllGathered router decisions ========
        # Inverse permutation (slot -> token id) built with matmuls instead of
        # 16 serialized indirect scatters: M[token, slot] = (rank == slot),
        # tok_list[slot] = sum_t M[t, slot] * t, with +BIG for empty slots.
        with tc.tile_pool(name="pe", bufs=1) as pe, \
             tc.tile_pool(name="pe2", bufs=3) as pe2, \
             tc.tile_pool(name="psE", bufs=2, space="PSUM") as psE, \
             tc.tile_pool(name="psE2", bufs=1, space="PSUM") as psE2:
            ut = pe.tile([P, P], F16)
            nc.sync.dma_start(ut[:], ex["ut_h"][:])
            slb = pe.tile([8, TB * P], F32R)
            nc.sync.dma_start(slb[:], ex["slb_in"][:].bitcast(F32R))
            s_iota = pe.tile([P, CAP], F32)
            nc.sync.dma_start(s_iota[:], ex["slot_iota"][:])
            tokid2 = pe.tile([P, 2 * TB], F16)
            nc.sync.dma_start(tokid2[:], ex["tokid2"][:])
            totals = pe.tile([8, E], F32R)
            pre_sb = [pe.tile([P, E], F32, tag=f"pre{b}", name=f"pre{b}")
                      for b in range(TB)]
            sel_all = [pe.tile([P, E], F16, tag=f"sela{b}", name=f"sela{b}")
                       for b in range(TB)]
            for b in range(TB):
                nc.sync.dma_start(
                    sel_all[b][:],
                    w_all[b * P:(b + 1) * P, E:2 * E])
                pr_ps = psE.tile([P, E], F32, tag="prps")
                nc.tensor.matmul(pr_ps[:], ut[:], sel_all[b][:],
                                 start=True, stop=True)
                nc.vector.tensor_copy(pre_sb[b][:], pr_ps[:])
                nc.sync.dma_start(totals[b:b + 1, :],
                                  pre_sb[b][127:128, :].bitcast(F32R))
            tl_ps = [[psE2.tile([P, 2], F32, tag=f"tl{ei}{ch}",
                                name=f"tl{ei}{ch}") for ch in range(2)]
                     for ei in range(2)]
            for b in range(TB):
                ofs_ps = psE.tile([P, E], F32, tag="ofsps", name="ofsps")
                nc.tensor.matmul(ofs_ps[:], slb[:, b * P:(b + 1) * P],
                                 totals[:], start=True, stop=True)
                grank = pe2.tile([P, E], F32, tag="grank")
                nc.vector.tensor_add(grank[:], pre_sb[b][:], ofs_ps[:])
                nc.vector.tensor_scalar_add(grank[:], grank[:], -1.0)
                gm = pe2.tile([P, E], F32, tag="gm")
                nc.vector.tensor_scalar(out=gm[:], in0=grank[:],
                                        scalar1=float(CAP - 1), scalar2=BIG,
                                        op0=ALU.is_gt, op1=ALU.mult)
                nc.vector.tensor_add(grank[:], grank[:], gm[:])
                um = pe2.tile([P, E], F32, tag="um")
                nc.vector.tensor_scalar(out=um[:], in0=sel_all[b][:],
                                        scalar1=-BIG, scalar2=BIG,
                                        op0=ALU.mult, op1=ALU.add)
                nc.vector.tensor_add(grank[:], grank[:], um[:])
                for ei in range(2):
                    ge = pe2.tile([P, E], F32, tag="ge")
                    nc.vector.tensor_mul(ge[:], grank[:],
                                         emask01[:, ei * E:(ei + 1) * E])
                    ridx = pe2.tile([P, 1], F32, tag="ridx")
                    nc.vector.reduce_sum(ridx[:], ge[:], axis=AX.X)
                    mb = pe2.tile([P, CAP], F16, tag="mb")
                    nc.vector.tensor_tensor(
                        out=mb[:], in0=s_iota[:],
                        in1=ridx[:, 0:1].to_broadcast([P, CAP]),
                        op=ALU.is_equal)
                    for ch in range(2):
                        nc.tensor.matmul(tl_ps[ei][ch][:],
                                         mb[:, ch * P:(ch + 1) * P],
                                         tokid2[:, 2 * b:2 * b + 2],
                                         start=(b == 0), stop=(b == TB - 1))
            for ei in range(2):
                for ch in range(2):
                    tl = pe2.tile([P, 2], F32, tag="tlsb")
                    nc.vector.tensor_copy(tl[:], tl_ps[ei][ch][:])
                    pad = pe2.tile([P, 1], F32, tag="pad")
                    nc.vector.tensor_scalar(out=pad[:], in0=tl[:, 1:2],
                                            scalar1=-BIG, scalar2=BIG,
                                            op0=ALU.mult, op1=ALU.add)
                    tok_f = pe2.tile([P, 1], F32, tag="tokf")
                    nc.vector.tensor_add(tok_f[:], tl[:, 0:1], pad[:])
                    nc.vector.tensor_copy(idx_sb2[ei][ch][:], tok_f[:])

        # ======== F: xT + shared expert + experts (fp16) ========
        with tc.tile_pool(name="pxt", bufs=1) as pxt, \
             tc.tile_pool(name="pfs", bufs=1) as pfs, \
             tc.tile_pool(name="pfs2", bufs=2) as pfs2:
            xc = [pxt.tile([P, T], F16, tag=f"xc{hc}", name=f"xc{hc}")
                  for hc in range(HC)]
            with tc.tile_pool(name="pxt2", bufs=3) as pxt2, \
                 tc.tile_pool(name="psX", bufs=3, space="PSUM") as psX:
                for b in range(TB):
                    xb = pxt2.tile([P, H], F16, tag="xb", bufs=3)
                    nc.sync.dma_start(xb[:], x_tm[b * P:(b + 1) * P, :])
                    for hc in range(HC):
                        tp = psX.tile([P, P], F16, tag="tpX")
                        nc.tensor.transpose(tp[:], xb[:, hc * P:(hc + 1) * P],
                                            identh[:])
                        if hc % 2 == 0:
                            nc.vector.tensor_copy(
                                xc[hc][:, b * P:(b + 1) * P], tp[:])
                        else:
                            nc.scalar.activation(
                                xc[hc][:, b * P:(b + 1) * P], tp[:], AF.Copy)

            # ---- both experts' setup: gathers, gxT, weights,
            # per-token gate weights — overlaps the shared expert below ----
            gxT2 = [pfs.tile([P, HC * 2 * P], F16, tag=f"gxT{ei}",
                             name=f"gxT{ei}") for ei in range(2)]
            wd_res2 = [[pfs.tile([P, H], F16, tag=f"wd{ei}{ip}",
                                 name=f"wd{ei}{ip}") for ip in range(IP)]
                       for ei in range(2)]
            wg_own2 = [[pfs.tile([P, 1], F32, tag=f"wgo{ei}{k}",
                                 name=f"wgo{ei}{k}") for k in range(2)]
                       for ei in range(2)]
            psS_cm = tc.tile_pool(name="psS", bufs=2, space="PSUM")
            psS = psS_cm.__enter__()
            for ei in range(2):
                for k in range(2):
                    gx = pfs2.tile([P, H], F16, tag="gx")
                    nc.vector.memset(gx[:], 0.0)
                    nc.gpsimd.indirect_dma_start(
                        out=gx[:], out_offset=None,
                        in_=x_tm[:],
                        in_offset=bass.IndirectOffsetOnAxis(
                            ap=idx_sb2[ei][k][:, :1], axis=0),
                        bounds_check=T - 1, oob_is_err=False)
                    for hc in range(HC):
                        tp = psS.tile([P, P], F16, tag="tpS")
                        nc.tensor.transpose(tp[:], gx[:, hc * P:(hc + 1) * P],
                                            identh[:])
                        dst = gxT2[ei][:, hc * 2 * P + k * P:
                                       hc * 2 * P + (k + 1) * P]
                        if hc % 2 == 0:
                            nc.vector.tensor_copy(dst, tp[:])
                        else:
                            nc.scalar.activation(dst, tp[:], AF.Copy)
                    wrow = pfs2.tile([P, 2 * E], F16, tag="wrow")
                    nc.vector.memset(wrow[:], 0.0)
                    nc.gpsimd.indirect_dma_start(
                        out=wrow[:], out_offset=None, in_=w_all[:],
                        in_offset=bass.IndirectOffsetOnAxis(
                            ap=idx_sb2[ei][k][:, :1], axis=0),
                        bounds_check=T - 1, oob_is_err=False)
                    we_ = pfs2.tile([P, E], F32, tag="we_")
                    nc.vector.tensor_mul(we_[:], wrow[:, 0:E],
                                         emask01[:, ei * E:(ei + 1) * E])
                    nc.vector.reduce_sum(wg_own2[ei][k][:], we_[:], axis=AX.X)

            # ---- shared expert ----
            with tc.tile_pool(name="pg", bufs=1) as pg, \
                 tc.tile_pool(name="pg2", bufs=3) as pg2:
                g_act = [pg.tile([P, T], F16, tag=f"gact{sp}", name=f"gact{sp}")
                         for sp in range(SP)]
                hs = [pg.tile([P, T], F16, tag=f"hs{sp}", name=f"hs{sp}")
                      for sp in range(SP)]
                with tc.tile_pool(name="psG1", bufs=1, space="PSUM") as psG1:
                    g_ps = [psG1.tile([P, T], F32, tag=f"gps{sp}",
                                      name=f"gps{sp}") for sp in range(SP)]
                    for hc in range(HC):
                        for sp in range(SP):
                            c0 = hc * SP * P + sp * P
                            for n in range(2):
                                sl = slice(n * 512, (n + 1) * 512)
                                nc.tensor.matmul(g_ps[sp][:, sl],
                                                 wsg_sb[:, c0:c0 + P],
                                                 xc[hc][:, sl],
                                                 start=(hc == 0),
                                                 stop=(hc == HC - 1))
                    for sp in range(SP):
                        nc.scalar.activation(g_act[sp][:], g_ps[sp][:],
                                             AF.Silu)
                with tc.tile_pool(name="psG2", bufs=1, space="PSUM") as psG2:
                    u_ps = [psG2.tile([P, T], F32, tag=f"ups{sp}",
                                      name=f"ups{sp}") for sp in range(SP)]
                    for hc in range(HC):
                        for sp in range(SP):
                            c0 = hc * SP * P + sp * P
                            for n in range(2):
                                sl = slice(n * 512, (n + 1) * 512)
                                nc.tensor.matmul(u_ps[sp][:, sl],
                                                 wsu_sb[:, c0:c0 + P],
                                                 xc[hc][:, sl],
                                                 start=(hc == 0),
                                                 stop=(hc == HC - 1))
                    for sp in range(SP):
                        nc.vector.tensor_mul(hs[sp][:], g_act[sp][:],
                                             u_ps[sp][:])
                with tc.tile_pool(name="psG3", bufs=6, space="PSUM") as psG3:
                    for tb_ in range(TB):
                        psd = [psG3.tile([P, 512], F32, tag="psGd",
                                         name=f"psGd{n}") for n in range(4)]
                        for sp in range(SP):
                            for n in range(4):
                                nc.tensor.matmul(
                                    psd[n][:],
                                    hs[sp][:, tb_ * P:(tb_ + 1) * P],
                                    wsd_sb[sp][:, n * 512:(n + 1) * 512],
                                    start=(sp == 0), stop=(sp == SP - 1))
                        sbd = pg2.tile([P, H], F16, tag="sbGd", bufs=2)
                        for n in range(4):
                            if n % 2 == 0:
                                nc.vector.tensor_copy(
                                    sbd[:, n * 512:(n + 1) * 512], psd[n][:])
                            else:
                                nc.scalar.activation(
                                    sbd[:, n * 512:(n + 1) * 512], psd[n][:],
                                    AF.Copy)
                        nc.sync.dma_start(rs2_in[tb_ * P:(tb_ + 1) * P, :],
                                          sbd[:])

            psS_cm.__exit__(None, None, None)

            # expert down-proj weights: emitted late so these 8 MB of DMAs
            # sit behind the x-block/gather traffic in queue priority, but
            # they still have ~100us of slack before first use
            for ei in range(2):
                for ip in range(IP):
                    nc.sync.dma_start(wd_res2[ei][ip][:],
                                      ex["we_d"][ei, ip * P:(ip + 1) * P, :])

            # ---- experts (setup already done above) ----
            for ei in range(2):
                with tc.tile_pool(name=f"pf{ei}", bufs=1) as pf, \
                     tc.tile_pool(name=f"pf2{ei}", bufs=2) as pf2:
                    idx_sb = idx_sb2[ei]
                    gxT = gxT2[ei]
                    wd_res = wd_res2[ei]

                    # merged gate+up pass (8 PSUM banks)
                    g_tm = [pf.tile([P, I], F16, tag=f"gtm{k}", name=f"gtm{k}")
                            for k in range(2)]
                    h_tm = [pf.tile([P, I], F16, tag=f"htm{k}", name=f"htm{k}")
                            for k in range(2)]
                    with tc.tile_pool(name=f"psF2{ei}", bufs=1,
                                      space="PSUM") as psF2:
                        gu_ps = [[psF2.tile([P, 512], F32, tag=f"gups{k}{j}",
                                            name=f"gups{k}{j}")
                                  for j in range(4)] for k in range(2)]
                        for hc in range(HC):
                            wg_c = pf2.tile([P, I], F16, tag="wgF", bufs=4)
                            nc.sync.dma_start(
                                wg_c[:], ex["we_g"][ei, hc * P:(hc + 1) * P, :])
                            wu_c = pf2.tile([P, I], F16, tag="wuF", bufs=4)
                            nc.sync.dma_start(
                                wu_c[:], ex["we_u"][ei, hc * P:(hc + 1) * P, :])
                            for k in range(2):
                                s_ = gxT[:, hc * 2 * P + k * P:
                                         hc * 2 * P + (k + 1) * P]
                                for n in range(2):
                                    nc.tensor.matmul(
                                        gu_ps[k][n][:], s_,
                                        wg_c[:, n * 512:(n + 1) * 512],
                                        start=(hc == 0), stop=(hc == HC - 1))
                                for n in range(2):
                                    nc.tensor.matmul(
                                        gu_ps[k][2 + n][:], s_,
                                        wu_c[:, n * 512:(n + 1) * 512],
                                        start=(hc == 0), stop=(hc == HC - 1))
                        for k in range(2):
                            for n in range(2):
                                sl = slice(n * 512, (n + 1) * 512)
                                nc.scalar.activation(g_tm[k][:, sl],
                                                     gu_ps[k][n][:], AF.Silu)
                                nc.vector.tensor_mul(h_tm[k][:, sl],
                                                     g_tm[k][:, sl],
                                                     gu_ps[k][2 + n][:])
                    h_sb = [pf.tile([P, 2 * P], F16, tag=f"hsb{ip}",
                                    name=f"hsb{ip}") for ip in range(IP)]
                    with tc.tile_pool(name=f"psF4{ei}", bufs=2,
                                      space="PSUM") as psF4:
                        for k in range(2):
                            for ip in range(IP):
                                tp = psF4.tile([P, P], F16, tag="tpF2")
                                nc.tensor.transpose(
                                    tp[:], h_tm[k][:, ip * P:(ip + 1) * P],
                                    identh[:])
                                nc.vector.tensor_copy(
                                    h_sb[ip][:, k * P:(k + 1) * P], tp[:])
                    with tc.tile_pool(name=f"psF5{ei}", bufs=8,
                                      space="PSUM") as psF5:
                        for k in range(2):
                            psd = [psF5.tile([P, 512], F32, tag="psFd",
                                             name=f"psFd{n}")
                                   for n in range(4)]
                            for ip in range(IP):
                                for n in range(4):
                                    nc.tensor.matmul(
                                        psd[n][:],
                                        h_sb[ip][:, k * P:(k + 1) * P],
                                        wd_res[ip][:, n * 512:(n + 1) * 512],
                                        start=(ip == 0), stop=(ip == IP - 1))
                            out_sb = pf.tile([P, H], F16, tag=f"outsb{k}")
                            for n in range(4):
                                o_ = out_sb[:, n * 512:(n + 1) * 512]
                                if n % 2 == 0:
                                    nc.vector.tensor_scalar_mul(
                                        o_, psd[n][:],
                                        wg_own2[ei][k][:, :1])
                                else:
                                    nc.scalar.activation(
                                        o_, psd[n][:], AF.Copy,
                                        scale=wg_own2[ei][k][:, :1])
                            nc.gpsimd.indirect_dma_start(
                                out=rs2_in[:],
                                out_offset=bass.IndirectOffsetOnAxis(
                                    ap=idx_sb[k][:, :1], axis=0),
                                in_=out_sb[:], in_offset=None,
                                bounds_check=T - 1, oob_is_err=False,
                                compute_op=ALU.add)

        nc.gpsimd.collective_compute(
            "ReduceScatter", ALU.add, ins=[rs2_in.opt()], outs=[rs2_out.opt()],
            replica_groups=[list(range(NCN))])
        with tc.tile_pool(name="pz", bufs=2) as pz:
            fin16 = pz.tile([P, H], F16)
            nc.sync.dma_start(fin16[:], rs2_out[:])
            fin = pz.tile([P, H], F32)
            nc.vector.tensor_copy(fin[:], fin16[:])
            nc.sync.dma_start(out_slice[:], fin[:])


_CACHE = {}


def _build():
    key = "nc"
    if key in _CACHE:
        return _CACHE[key]
    nc = bacc.Bacc("TRN2", target_bir_lowering=False, debug=False,
                   num_devices=NCN)
    with tile.TileContext(nc) as tc:
        _emit(nc, tc)
    nc.compile()
    _CACHE[key] = nc
    return nc


def _host_prep(inputs):
    f16 = np.float16
    pos = np.asarray(inputs["positions"]).astype(np.float64)
    hid = np.asarray(inputs["hidden_states"], np.float32)
    w_in = np.asarray(inputs["w_in_ln"], np.float32)
    w_post = np.asarray(inputs["w_post_ln"], np.float32)
    wq = (np.asarray(inputs["wq"], np.float32) * w_in[:, None]).astype(f16)
    wk = (np.asarray(inputs["wk"], np.float32) * w_in[:, None]).astype(f16)
    wv = (np.asarray(inputs["wv"], np.float32) * w_in[:, None]).astype(f16)
    wo = np.asarray(inputs["wo"], np.float32).astype(f16)
    gate_w = np.asarray(inputs["gate_w"], np.float32) * w_post[None, :]
    gate_b = np.asarray(inputs["gate_bias"], np.float32).reshape(1, E)
    we_g = (np.asarray(inputs["we_gate"], np.float32)
            * w_post[None, :, None]).astype(f16)
    we_u = (np.asarray(inputs["we_up"], np.float32)
            * w_post[None, :, None]).astype(f16)
    we_d = np.asarray(inputs["we_down"], np.float32).astype(f16)
    ws_g = (np.asarray(inputs["ws_gate"], np.float32)
            * w_post[:, None]).astype(f16)
    ws_u = (np.asarray(inputs["ws_up"], np.float32)
            * w_post[:, None]).astype(f16)
    ws_d = np.asarray(inputs["ws_down"], np.float32).astype(f16)

    inv_freq = 1.0 / (THETA ** (np.arange(0, D, 2, dtype=np.float64) / D))
    f = pos[None, :] * inv_freq[:, None]
    cos2, sin2 = np.cos(f), np.sin(f)
    cosT = np.repeat(cos2, 2, axis=0).astype(np.float32)
    sinT = np.empty((D, T), np.float32)
    sinT[0::2] = -sin2
    sinT[1::2] = sin2
    s = 1.0 / np.sqrt(D)
    cosq, sinq = (cosT * s).astype(np.float32), (sinT * s).astype(np.float32)

    import ml_dtypes
    bf = ml_dtypes.bfloat16
    ii = np.arange(P)
    diag_mask = np.where(ii[:, None] >= ii[None, :], 0.0, NEG).astype(bf)

    identr_in = np.eye(P, dtype=np.float32)
    identh_in = np.eye(P, dtype=f16)
    ut_in = np.triu(np.ones((P, P), np.float32))
    slb_in = np.zeros((8, TB * P), np.float32)
    for b in range(TB):
        slb_in[:b, b * P:(b + 1) * P] = 1.0
    perm = np.zeros((P, P), np.float32)
    for i in range(0, P, 2):
        perm[i, i + 1] = 1.0
        perm[i + 1, i] = 1.0
    slot_iota = np.broadcast_to(np.arange(CAP, dtype=np.float32),
                                (P, CAP)).copy()
    tokid2 = np.zeros((P, 2 * TB), f16)
    for b in range(TB):
        tokid2[:, 2 * b] = (b * P + np.arange(P)).astype(f16)
        tokid2[:, 2 * b + 1] = 1.0

    ISC = IS // NCN
    maps = []
    for c in range(NCN):
        g = c // 2
        emask01 = np.zeros((P, 2 * E), np.float32)
        emask01[:, 2 * c] = 1.0          # ei = 0 -> expert 2c
        emask01[:, E + 2 * c + 1] = 1.0  # ei = 1 -> expert 2c+1
        maps.append({
            "hid": hid,
            "hid_slice": np.ascontiguousarray(
                hid[PERM_ALL[c * P:(c + 1) * P]]),
            "wq_s": np.ascontiguousarray(wq[:, 2 * c * D:(2 * c + 2) * D]),
            "wk_s": np.ascontiguousarray(wk[:, g * D:(g + 1) * D]),
            "wv_s": np.ascontiguousarray(wv[:, g * D:(g + 1) * D]),
            "wo_s": np.ascontiguousarray(wo[2 * c * D:(2 * c + 2) * D, :]),
            "cosq": cosq, "sinq": sinq, "cosk": cosT, "sink": sinT,
            "perm": perm, "diag_mask": diag_mask,
            "identr_in": identr_in, "identh_in": identh_in,
            "ut_in": ut_in, "ut_h": ut_in.astype(f16), "slb_in": slb_in,
            "slot_iota": slot_iota, "tokid2": tokid2,
            "gate_wT": np.ascontiguousarray(gate_w.T),
            "gate_b": np.broadcast_to(gate_b, (P, E)).copy(),
            "emask01": emask01,
            "ws_g": np.ascontiguousarray(ws_g[:, c * ISC:(c + 1) * ISC]),
            "ws_u": np.ascontiguousarray(ws_u[:, c * ISC:(c + 1) * ISC]),
            "ws_d": np.ascontiguousarray(ws_d[c * ISC:(c + 1) * ISC, :]),
            "we_g": np.ascontiguousarray(we_g[2 * c:2 * c + 2]),
            "we_u": np.ascontiguousarray(we_u[2 * c:2 * c + 2]),
            "we_d": np.ascontiguousarray(we_d[2 * c:2 * c + 2]),
        })
    return maps


def kernel(trace=False, **inputs):
    nc = _build()
    maps = _host_prep(inputs)
    res = bass_utils.run_bass_kernel_spmd(
        nc, maps, core_ids=list(range(NCN)), trace=trace)
    out = np.empty((T, H), np.float32)
    out[PERM_ALL] = np.concatenate(
        [res.results[c]["out_slice"] for c in range(NCN)], 0)
    resid = np.empty((T, H), np.float32)
    resid[PERM_ALL] = np.concatenate(
        [res.results[c]["res_slice"] for c in range(NCN)], 0)
    kernel.last_results = res
    return out, resid
